# revision 7
# baseline (speedup 1.0000x reference)
"""GAT 2-layer GNN on 8 Trainium2 NeuronCores (Bass/Tile SPMD).

Strategy (BNS-GCN style node partition):
- Nodes sharded 8 ways by contiguous ranges of 12500 (dst ownership).
- Edges assigned to the owner of their dst, sorted by dst, grouped into
  128-node "windows" (98 per core); each window's edge list is padded to a
  multiple of 128 (edge tiles). Tile counts per window are equalized across
  cores so all 8 cores run one SPMD program.
- Gather tables are packed per-rank shards of 12544 rows, each row
  [el(2) | er(2) | x(FIN)], built by a sharded node pass + AllGather.
  Layer 0: x = feat (128 cols); layer 1: x = h (64 cols).
- Per edge tile [128 edges]: ONE indirect-DMA gather of source rows
  (~1us GpSimd issue each -- the hard floor on this ISA: one index per
  partition per instruction). er[dst] comes from a small PE matmul against
  the window's own-er columns with a transposed one-hot built on DVE from
  DMA-replicated ld values. ee = exp(leaky_relu(el+er)) on ACT. Two
  ee-weighted one-hots (fused DVE op each) drive per-head PE matmuls
  accumulating [num | den] into PSUM per window (den via an always-1.0
  SBUF column adjacent to the gathered x).
- Per window [128 dst nodes]: out_h = (num_h @ W_h)/den_h, head-mean + bias,
  LayerNorm+ReLU (layer 0), h/el2/er2 shard row write; final layer writes out.
- Softmax uses exp without max subtraction (logits bounded ~|6|, safe in
  f32); num/den division after aggregation is mathematically identical to
  the reference's per-edge alpha normalization.
"""
import os
import numpy as np

import concourse.bass as bass
import concourse.bacc as bacc
import concourse.mybir as mybir
from concourse.bass_utils import run_bass_kernel_spmd
from concourse.tile import TileContext
from concourse.masks import make_identity

F32 = mybir.dt.float32
I32 = mybir.dt.int32
AF = mybir.ActivationFunctionType
ALU = mybir.AluOpType
P = 128
SLOPE = 0.2
EPS = 1e-5

N_NODES = 100000
N_CORES = 8
OWN = 12500
NWIN = 98            # ceil(12500/128)
SHARD = NWIN * P     # 12544
GTAB = N_CORES * SHARD
GB = 16              # tiles per ldrep/ohT batch
NSX = 8              # gather slots
NEG = -1.0e4         # pad logit; exp(0.2*(NEG+NEG)) == 0 in f32

_cache = {}


def _schedule(src, dst):
    """Per-core edge slot assignment. Returns (Tw tuple, percore list of dicts)."""
    src = src.astype(np.int64)
    dst = dst.astype(np.int64)
    core = dst // OWN
    loc = dst - core * OWN
    win = loc >> 7
    lrel = (loc - (win << 7)).astype(np.float32)

    cnt = np.zeros((N_CORES, NWIN), np.int64)
    np.add.at(cnt, (core, win), 1)
    Tw = np.maximum(1, -(-cnt // P)).max(axis=0)
    T = int(Tw.sum())
    pad = (-T) % GB
    if pad:
        Tw[NWIN - 1] += pad
    T = int(Tw.sum())
    off = np.zeros(NWIN, np.int64)
    off[1:] = np.cumsum(Tw)[:-1]

    order = np.lexsort((src, win, core))
    so, co, wo, lo = src[order], core[order], win[order], lrel[order]

    percore = []
    for d in range(N_CORES):
        m = co == d
        sd_, wd_, ld_ = so[m], wo[m], lo[m]
        padrow = d * SHARD + OWN
        nslot = T * P
        sg = np.full(nslot, padrow, np.int32)   # packed src row
        lr = np.zeros(nslot, np.float32)
        wcnt = np.bincount(wd_.astype(np.int64), minlength=NWIN)
        pos = np.repeat(off * P, wcnt)
        iw = np.arange(len(wd_)) - np.repeat(
            np.concatenate(([0], np.cumsum(wcnt)[:-1])), wcnt
        )
        slots = pos + iw
        sg[slots] = ((sd_ // OWN) * SHARD + sd_ % OWN).astype(np.int32)
        lr[slots] = ld_
        sgm = sg.reshape(T, P)
        lrm = lr.reshape(T, P)
        percore.append(
            dict(
                sg=np.ascontiguousarray(sgm.T),      # [128, T] column-per-tile
                lrT=np.ascontiguousarray(lrm.T),     # [128, T]
                lrR=np.ascontiguousarray(lrm),       # [T, 128] row-per-tile
            )
        )
    return tuple(int(x) for x in Tw), percore


def _build(Tw):
    T = int(sum(Tw))
    nc = bacc.Bacc(None, target_bir_lowering=False, debug=False)

    feat_own = nc.dram_tensor("feat_own", [SHARD, P], F32, kind="ExternalInput")
    sg_in = nc.dram_tensor("sg", [P, T], I32, kind="ExternalInput")
    lrT_in = nc.dram_tensor("lrT", [P, T], F32, kind="ExternalInput")
    lrR_in = nc.dram_tensor("lrR", [T, P], F32, kind="ExternalInput")
    iota_in = nc.dram_tensor("iota", [P, P], F32, kind="ExternalInput")
    iotac_in = nc.dram_tensor("iotac", [P, 1], F32, kind="ExternalInput")
    W1_in = nc.dram_tensor("W1", [P, P], F32, kind="ExternalInput")
    W2_in = nc.dram_tensor("W2", [64, P], F32, kind="ExternalInput")
    wlr1_in = nc.dram_tensor("wlr1", [P, 4], F32, kind="ExternalInput")
    wlr2_in = nc.dram_tensor("wlr2", [64, 4], F32, kind="ExternalInput")
    bm1_in = nc.dram_tensor("bm1", [P, 64], F32, kind="ExternalInput")
    gm_in = nc.dram_tensor("gm", [P, 64], F32, kind="ExternalInput")
    bt_in = nc.dram_tensor("bt", [P, 64], F32, kind="ExternalInput")
    bm2_in = nc.dram_tensor("bm2", [P, 64], F32, kind="ExternalInput")
    out_ext = nc.dram_tensor("out", [OWN, 64], F32, kind="ExternalOutput")

    t1_sh = nc.dram_tensor("t1_sh", [SHARD, 132], F32)
    t1_full = nc.dram_tensor("t1_full", [GTAB, 132], F32, addr_space="Shared")
    t2_sh = nc.dram_tensor("t2_sh", [SHARD, 68], F32)
    t2_full = nc.dram_tensor("t2_full", [GTAB, 68], F32, addr_space="Shared")

    RG = [list(range(N_CORES))]

    with TileContext(nc) as tc:
        with (
            tc.tile_pool(name="const", bufs=1) as cp,
            tc.tile_pool(name="slots", bufs=1) as sp,
            tc.tile_pool(name="work", bufs=4) as wp,
            tc.tile_pool(name="grp", bufs=2) as gp,
            tc.tile_pool(name="win", bufs=2) as wn,
            tc.tile_pool(name="psa", bufs=2, space="PSUM") as psa,
            tc.tile_pool(name="pse", bufs=2, space="PSUM") as pse,
            tc.tile_pool(name="psb", bufs=2, space="PSUM") as psb,
        ):
            sgT = cp.tile([P, T], I32)
            nc.sync.dma_start(out=sgT[:], in_=sg_in[:])
            lrT = cp.tile([P, T], F32)
            nc.sync.dma_start(out=lrT[:], in_=lrT_in[:])
            iota = cp.tile([P, P], F32)
            nc.sync.dma_start(out=iota[:], in_=iota_in[:])
            iotac = cp.tile([P, 1], F32)
            nc.sync.dma_start(out=iotac[:], in_=iotac_in[:])
            W1sb = cp.tile([P, P], F32)
            nc.sync.dma_start(out=W1sb[:], in_=W1_in[:])
            W2sb = cp.tile([64, P], F32)
            nc.sync.dma_start(out=W2sb[:], in_=W2_in[:])
            wlr1 = cp.tile([P, 4], F32)
            nc.sync.dma_start(out=wlr1[:], in_=wlr1_in[:])
            wlr2 = cp.tile([64, 4], F32)
            nc.sync.dma_start(out=wlr2[:], in_=wlr2_in[:])
            bm1 = cp.tile([P, 64], F32)
            nc.sync.dma_start(out=bm1[:], in_=bm1_in[:])
            gm = cp.tile([P, 64], F32)
            nc.sync.dma_start(out=gm[:], in_=gm_in[:])
            bt = cp.tile([P, 64], F32)
            nc.sync.dma_start(out=bt[:], in_=bt_in[:])
            bm2 = cp.tile([P, 64], F32)
            nc.sync.dma_start(out=bm2[:], in_=bm2_in[:])
            ident = cp.tile([P, P], F32)
            make_identity(nc, ident[:])
            eps_t = cp.tile([P, 1], F32)
            nc.vector.memset(eps_t[:], EPS)
            erown1 = cp.tile([P, 2 * NWIN], F32)
            erown2 = cp.tile([P, 2 * NWIN], F32)

            # ---- phase B: build layer-0 table shard rows [el1|er1|x] ----
            with nc.named_scope("tab1"):
                for wi in range(NWIN):
                    xt = wp.tile([P, 132], F32, tag="xt", name=f"xt{wi}")
                    nc.sync.dma_start(
                        out=xt[:, 4:132], in_=feat_own[wi * P : (wi + 1) * P, :]
                    )
                    pT = psb.tile([P, P], F32, tag="pst", name=f"pb{wi}")
                    nc.tensor.transpose(out=pT[:], in_=xt[:, 4:132], identity=ident[:])
                    xT = wp.tile([P, P], F32, tag="xT", name=f"xT{wi}")
                    nc.vector.tensor_copy(out=xT[:], in_=pT[:])
                    pE = psb.tile([P, 64], F32, tag="pst", name=f"pe{wi}")
                    nc.tensor.matmul(
                        out=pE[:, 0:4], lhsT=xT[:], rhs=wlr1[:], start=True, stop=True
                    )
                    nc.vector.tensor_copy(out=xt[:, 0:4], in_=pE[:, 0:4])
                    nc.vector.tensor_copy(
                        out=erown1[:, 2 * wi : 2 * wi + 2], in_=pE[:, 2:4]
                    )
                    rows = min(P, OWN - wi * P)
                    nc.sync.dma_start(
                        out=t1_sh[wi * P : wi * P + rows, :], in_=xt[:rows, :]
                    )
                padt = cp.tile([SHARD - OWN, 132], F32)
                nc.vector.memset(padt[:], 0.0)
                nc.vector.memset(padt[:, 0:4], NEG)
                nc.sync.dma_start(out=t1_sh[OWN:SHARD, :], in_=padt[:])
                nc.gpsimd.collective_compute(
                    "AllGather", ALU.bypass, replica_groups=RG,
                    ins=[t1_sh[:]], outs=[t1_full[:]],
                )

            xs1 = [sp.tile([P, 133], F32, name=f"xs1_{j}") for j in range(NSX)]
            xs2 = [sp.tile([P, 69], F32, name=f"xs2_{j}") for j in range(NSX)]
            for t_ in xs1 + xs2:
                nc.vector.memset(t_[:], 1.0)

            def edge_layer(layer):
                if layer == 0:
                    tab, FIN, Wsb, erown, bm = t1_full, P, W1sb, erown1, bm1
                else:
                    tab, FIN, Wsb, erown, bm = t2_full, 64, W2sb, erown2, bm2
                CB = FIN + 5  # slot cols: [el(2), er(2), x(FIN), ones(1)]
                xsl = xs1 if layer == 0 else xs2
                ldrep = None
                ohT_all = None

                t = 0
                for w in range(NWIN):
                    nt = Tw[w]
                    pa = [
                        psa.tile(
                            [P, FIN + 1], F32, tag=f"agg{h}",
                            name=f"pa{layer}_{w}_{h}",
                        )
                        for h in range(2)
                    ]
                    for j in range(nt):
                        if t % GB == 0:
                            g_ = t // GB
                            ldrep = gp.tile(
                                [P, GB * P], F32, tag="ldrep", name=f"ldr{layer}_{g_}"
                            )
                            nc.sync.dma_start(
                                out=ldrep[:],
                                in_=lrR_in[g_ * GB : (g_ + 1) * GB, :]
                                .rearrange("t p -> (t p)")[None, :]
                                .to_broadcast([P, GB * P]),
                            )
                            ohT_all = gp.tile(
                                [P, GB * P], F32, tag="ohT", name=f"ohT{layer}_{g_}"
                            )
                            nc.vector.tensor_scalar(
                                out=ohT_all[:], in0=ldrep[:], scalar1=iotac[:],
                                scalar2=None, op0=ALU.is_equal,
                            )
                        slot = xsl[t % NSX]
                        nc.gpsimd.indirect_dma_start(
                            out=slot[:, 0 : CB - 1],
                            out_offset=None,
                            in_=tab[:],
                            in_offset=bass.IndirectOffsetOnAxis(
                                ap=sgT[:, t : t + 1], axis=0
                            ),
                        )
                        tk = t % GB
                        pe_ = pse.tile([P, 2], F32, tag="per", name=f"per{layer}_{t}")
                        nc.tensor.matmul(
                            out=pe_[:],
                            lhsT=ohT_all[:, tk * P : (tk + 1) * P],
                            rhs=erown[:, 2 * w : 2 * w + 2],
                            start=True,
                            stop=True,
                        )
                        ts_ = wp.tile([P, 2], F32, tag="ts", name=f"ts{layer}_{t}")
                        nc.vector.tensor_tensor(
                            out=ts_[:], in0=slot[:, 0:2], in1=pe_[:], op=ALU.add
                        )
                        nc.vector.scalar_tensor_tensor(
                            out=ts_[:], in0=ts_[:], scalar=SLOPE, in1=ts_[:],
                            op0=ALU.mult, op1=ALU.max,
                        )
                        ee = wp.tile([P, 2], F32, tag="ee", name=f"ee{layer}_{t}")
                        nc.scalar.activation(out=ee[:], in_=ts_[:], func=AF.Exp)
                        for h in range(2):
                            oh = wp.tile(
                                [P, P], F32, tag=f"oh{h}", name=f"oh{layer}_{t}_{h}"
                            )
                            nc.vector.scalar_tensor_tensor(
                                out=oh[:],
                                in0=iota[:],
                                scalar=lrT[:, t : t + 1],
                                in1=ee[:, h : h + 1].to_broadcast([P, P]),
                                op0=ALU.is_equal,
                                op1=ALU.mult,
                            )
                            nc.tensor.matmul(
                                out=pa[h][:],
                                lhsT=oh[:],
                                rhs=slot[:, 4:CB],
                                start=(j == 0),
                                stop=(j == nt - 1),
                            )
                        t += 1

                    # ---- window transform ----
                    rows = min(P, OWN - w * P)
                    den = wn.tile([P, 2], F32, tag="den", name=f"dn{layer}_{w}")
                    for h in range(2):
                        nc.vector.tensor_copy(
                            out=den[:, h : h + 1], in_=pa[h][:, FIN : FIN + 1]
                        )
                    nc.vector.tensor_scalar(
                        out=den[:], in0=den[:], scalar1=1e-30, scalar2=None,
                        op0=ALU.max,
                    )
                    rden = wn.tile([P, 2], F32, tag="rden", name=f"rd{layer}_{w}")
                    nc.vector.reciprocal(out=rden[:], in_=den[:])
                    o_ = []
                    for h in range(2):
                        nsb = wn.tile(
                            [P, FIN], F32, tag=f"nsb{h}", name=f"nb{layer}_{w}_{h}"
                        )
                        nc.scalar.activation(
                            out=nsb[:], in_=pa[h][:, 0:FIN], func=AF.Copy
                        )
                        pT = psb.tile([P, P], F32, tag="pst", name=f"pt{layer}_{w}_{h}")
                        nc.tensor.transpose(
                            out=pT[:FIN, :], in_=nsb[:], identity=ident[:]
                        )
                        nT = wn.tile([P, P], F32, tag=f"nT{h}", name=f"nt{layer}_{w}_{h}")
                        nc.vector.tensor_copy(out=nT[:FIN, :], in_=pT[:FIN, :])
                        po = psb.tile([P, 64], F32, tag="pst", name=f"po{layer}_{w}_{h}")
                        nc.tensor.matmul(
                            out=po[:],
                            lhsT=nT[:FIN, :],
                            rhs=Wsb[:, h * 64 : (h + 1) * 64],
                            start=True,
                            stop=True,
                        )
                        ov = wn.tile([P, 64], F32, tag=f"ov{h}", name=f"ov{layer}_{w}_{h}")
                        nc.vector.tensor_scalar(
                            out=ov[:], in0=po[:], scalar1=rden[:, h : h + 1],
                            scalar2=None, op0=ALU.mult,
                        )
                        o_.append(ov)
                    ssum = wn.tile([P, 64], F32, tag="ssum", name=f"ss{layer}_{w}")
                    nc.vector.tensor_tensor(
                        out=ssum[:], in0=o_[0][:], in1=o_[1][:], op=ALU.add
                    )
                    if layer == 1:
                        hm = wn.tile([P, 64], F32, tag="hm", name=f"hm{layer}_{w}")
                        nc.vector.scalar_tensor_tensor(
                            out=hm[:], in0=ssum[:], scalar=0.5, in1=bm[:],
                            op0=ALU.mult, op1=ALU.add,
                        )
                        nc.sync.dma_start(
                            out=out_ext[w * P : w * P + rows, :], in_=hm[:rows, :]
                        )
                        continue
                    hrow = wn.tile([P, 68], F32, tag="hrow", name=f"hr{w}")
                    nc.vector.scalar_tensor_tensor(
                        out=hrow[:, 4:68], in0=ssum[:], scalar=0.5, in1=bm[:],
                        op0=ALU.mult, op1=ALU.add,
                    )
                    # LayerNorm + ReLU on hrow[:, 4:68]
                    hm = hrow[:, 4:68]
                    mu = wn.tile([P, 1], F32, tag="mu", name=f"mu{w}")
                    nc.vector.reduce_sum(out=mu[:], in_=hm, axis=mybir.AxisListType.X)
                    nc.vector.tensor_scalar(
                        out=mu[:], in0=mu[:], scalar1=1.0 / 64, scalar2=None,
                        op0=ALU.mult,
                    )
                    xc = wn.tile([P, 64], F32, tag="xc", name=f"xc{w}")
                    nc.vector.tensor_scalar(
                        out=xc[:], in0=hm, scalar1=mu[:], scalar2=None,
                        op0=ALU.subtract,
                    )
                    sq = wn.tile([P, 64], F32, tag="sq", name=f"sq{w}")
                    ssq = wn.tile([P, 1], F32, tag="ssq", name=f"sv{w}")
                    nc.scalar.activation(
                        out=sq[:], in_=xc[:], func=AF.Square, accum_out=ssq[:]
                    )
                    sd_ = wn.tile([P, 1], F32, tag="sd", name=f"sd{w}")
                    nc.scalar.activation(
                        out=sd_[:], in_=ssq[:], func=AF.Sqrt, bias=eps_t[:],
                        scale=1.0 / 64,
                    )
                    rstd = wn.tile([P, 1], F32, tag="rstd", name=f"rs{w}")
                    nc.vector.reciprocal(out=rstd[:], in_=sd_[:])
                    nc.vector.tensor_scalar(
                        out=xc[:], in0=xc[:], scalar1=rstd[:], scalar2=None,
                        op0=ALU.mult,
                    )
                    nc.vector.tensor_tensor(out=xc[:], in0=xc[:], in1=gm[:], op=ALU.mult)
                    nc.vector.tensor_tensor(out=xc[:], in0=xc[:], in1=bt[:], op=ALU.add)
                    nc.scalar.activation(out=hm, in_=xc[:], func=AF.Relu)
                    # el2/er2 = h @ wlr2
                    pT2 = psb.tile([P, P], F32, tag="pst", name=f"p2{w}")
                    nc.tensor.transpose(out=pT2[:64, :], in_=hm, identity=ident[:])
                    hT = wn.tile([64, P], F32, tag="hT", name=f"ht{w}")
                    nc.vector.tensor_copy(out=hT[:], in_=pT2[:64, :])
                    pE2 = psb.tile([P, 64], F32, tag="pst", name=f"q2{w}")
                    nc.tensor.matmul(
                        out=pE2[:, 0:4], lhsT=hT[:], rhs=wlr2[:], start=True, stop=True
                    )
                    nc.vector.tensor_copy(out=hrow[:, 0:4], in_=pE2[:, 0:4])
                    nc.vector.tensor_copy(
                        out=erown2[:, 2 * w : 2 * w + 2], in_=pE2[:, 2:4]
                    )
                    nc.sync.dma_start(
                        out=t2_sh[w * P : w * P + rows, :], in_=hrow[:rows, :]
                    )

            with nc.named_scope("layer0"):
                edge_layer(0)
                padh = cp.tile([SHARD - OWN, 68], F32)
                nc.vector.memset(padh[:], 0.0)
                nc.vector.memset(padh[:, 0:4], NEG)
                nc.sync.dma_start(out=t2_sh[OWN:SHARD, :], in_=padh[:])
            with nc.named_scope("ag_h"):
                nc.gpsimd.collective_compute(
                    "AllGather", ALU.bypass, replica_groups=RG,
                    ins=[t2_sh[:]], outs=[t2_full[:]],
                )
            with nc.named_scope("layer1"):
                edge_layer(1)

    nc.compile()
    return nc


def kernel(
    feat, src, dst, W1, al1, ar1, b1, gamma1, beta1, W2, al2, ar2, b2, **extra
):
    feat = np.ascontiguousarray(feat, np.float32)
    Tw, percore = _schedule(np.asarray(src), np.asarray(dst))
    if Tw not in _cache:
        _cache[Tw] = _build(Tw)
    nc = _cache[Tw]

    W1 = np.asarray(W1, np.float32)
    W2 = np.asarray(W2, np.float32)
    wl1 = np.stack([W1[:, h * 64 : (h + 1) * 64] @ np.asarray(al1, np.float32)[h] for h in range(2)], 1)
    wr1 = np.stack([W1[:, h * 64 : (h + 1) * 64] @ np.asarray(ar1, np.float32)[h] for h in range(2)], 1)
    wlr1 = np.concatenate([wl1, wr1], axis=1).astype(np.float32)  # [128,4]
    wl2 = np.stack([W2[:, h * 64 : (h + 1) * 64] @ np.asarray(al2, np.float32)[h] for h in range(2)], 1)
    wr2 = np.stack([W2[:, h * 64 : (h + 1) * 64] @ np.asarray(ar2, np.float32)[h] for h in range(2)], 1)
    wlr2 = np.concatenate([wl2, wr2], axis=1).astype(np.float32)  # [64,4]
    b1 = np.asarray(b1, np.float32).reshape(2, 64)
    bm1 = np.tile(b1.mean(0), (P, 1)).astype(np.float32)
    b2 = np.asarray(b2, np.float32).reshape(2, 64)
    bm2 = np.tile(b2.mean(0), (P, 1)).astype(np.float32)
    gm = np.tile(np.asarray(gamma1, np.float32), (P, 1))
    bt = np.tile(np.asarray(beta1, np.float32), (P, 1))
    iota = np.tile(np.arange(P, dtype=np.float32), (P, 1))
    iotac = np.arange(P, dtype=np.float32).reshape(P, 1)

    in_maps = []
    for d in range(N_CORES):
        po = percore[d]
        fo = np.zeros((SHARD, P), np.float32)
        fo[:OWN] = feat[d * OWN : (d + 1) * OWN]
        in_maps.append(
            dict(
                feat_own=fo, sg=po["sg"], lrT=po["lrT"], lrR=po["lrR"],
                iota=iota, iotac=iotac, W1=W1, W2=W2, wlr1=wlr1, wlr2=wlr2,
                bm1=bm1, gm=gm, bt=bt, bm2=bm2,
            )
        )

    trace = bool(os.environ.get("GAT_TRACE"))
    res = run_bass_kernel_spmd(nc, in_maps, list(range(N_CORES)), trace=trace)
    kernel.last_result = res
    out = np.concatenate([r["out"] for r in res.results], axis=0)
    return out
